# revision 34
# baseline (speedup 1.0000x reference)
"""Trainium2 Bass kernel for nn_MixtureOfExpertsModel (moe_routing).

Computes, for x [65536, 1024] and 10 experts with 15 outputs each:
    miu = x @ expert_w^T + expert_b      (per expert)
    xi  = x @ gate_w^T + gate_b          (per expert)
    out = sum_e softmax_e(xi) * miu      -> [65536, 15]

Strategy: pure data parallel over 8 NeuronCores (8192 rows each); at fp16
the kernel is PE-bound (2.52 GMAC/core -> 153.6k PE cycles = 64 us), so the
design keeps the PE streaming 300-column fp16 matmuls back-to-back at the
~127 ns issue floor and pushes everything else off the critical path:

 * x is repacked on the host into per-slab blocks (512 rows; one contiguous
   8KB run per partition) so every HWDGE load is 128 large descriptors --
   the SDMA per-descriptor fixed cost (~130ns/engine) makes smaller
   descriptors drain far below line rate.
 * head: the first real matmul needs the weights and slab 0, and the Sync
   HWDGE ring starts draining ~2us before the Scalar ring.  Weights and
   slab0's first half ride Sync, slab0's second half plus the (tiny, fp16,
   broadcast on-chip) bias ride Scalar, so group 0 starts ~14us; PE warmup
   matmuls hold the HAM clock gate open until then (a >3.4us idle gap
   would drop the PE back to 1.2GHz for ~3.4us of real matmuls).
 * no bias matmul: the psum->SBUF eviction is a single Vector tensor_add
   per slab that adds the bias and narrows to fp16 (sole PSUM reader).
 * post-processing is batched per 4-subtile slab with h-major planes
   (mx/pe = [p, h, s, 150]) so exp (Scalar) and the product (Vector fp16
   2x) run on contiguous [p, 600] planes; segmented reduce over experts
   (Vector; tensor_reduce has no DVE 2x mode); reciprocal per output GROUP
   (Vector) and the final num*rden on the otherwise idle GpSimd (except
   the last group: Vector, for a shorter tail chain).
 * the last TWO slabs run as two half-slab psum tiles each with per-half
   post chains, halving the serial chain after the final matmul; rows are
   permuted (within each 2048-row group, partition p owns rows
   p*16..p*16+15) so each output store is 128 x 960B descriptors instead
   of 2048 x 60B, keeping the store off the critical tail.
"""

import sys

if "/opt/trn_rl_repo" not in sys.path:
    sys.path.insert(0, "/opt/trn_rl_repo")

import numpy as np

import concourse.bass as bass
import concourse.bacc as bacc
import concourse.tile as tile
import concourse.mybir as mybir
from concourse.bass_utils import run_bass_kernel_spmd

F32 = mybir.dt.float32
FP16 = mybir.dt.float16
BF16 = mybir.dt.bfloat16

MDT = FP16
NPDT = np.float16

BS = 65536
K = 1024
E = 10
O = 15
EO = E * O                # 150
NCOL = 2 * EO             # 300: cols 0..149 = expert (n=o*E+e), 150..299 = gate
NCORES = 8
RPC = BS // NCORES        # rows per core: 8192
KC = K // 128             # 8 contraction chunks
SLAB = 512                # rows per slab = 4 matmul subtiles
NSUB = SLAB // 128        # 4 subtiles per slab
NSLAB = RPC // SLAB       # 16 slabs per core
GROUP = 4                 # slabs per output group (2048 rows per out DMA)
NGRP = NSLAB // GROUP
PREFETCH = 5              # x slabs in flight ahead of compute
N_WARMUP = 32             # PE warmup matmuls (HAM clock-gate release)
FINE = (NSLAB - 2, NSLAB - 1)   # slabs split for a short tail


def _build():
    nc = bacc.Bacc("TRN2", target_bir_lowering=False, debug=False,
                   num_devices=NCORES)
    # xt row k*128+q holds slab k's contiguous (j, c, m) block: j = subtile,
    # c = k-chunk, m = moving-row index p; q = k-chunk partition.
    xt = nc.dram_tensor("xt", [NSLAB * 128, NSUB * KC * 128], MDT,
                        kind="ExternalInput").ap()
    wt = nc.dram_tensor("wt", [128, KC * NCOL], MDT, kind="ExternalInput").ap()
    biasb = nc.dram_tensor("biasb", [128, NCOL], MDT,
                           kind="ExternalInput").ap()
    out = nc.dram_tensor("out", [RPC, O], F32, kind="ExternalOutput").ap()

    HKC = KC // 2
    HX = 2 * KC * 128     # half-slab elements per partition

    with tile.TileContext(nc) as tc:
        with (
            tc.tile_pool(name="const", bufs=1) as cp,
            tc.tile_pool(name="x0", bufs=1) as x0p,
            tc.tile_pool(name="x", bufs=PREFETCH + 2) as xp,
            tc.tile_pool(name="ps", bufs=2, space="PSUM") as ps_pool,
            tc.tile_pool(name="mx", bufs=2) as mx_pool,
            tc.tile_pool(name="pe", bufs=2) as pe_pool,
            tc.tile_pool(name="nd", bufs=2) as nd_pool,
            tc.tile_pool(name="ob", bufs=2) as ob_pool,
        ):
            # Sync ring (drains earliest): weights, slab0 subtiles 0-1, then
            # the slab stream.  Scalar ring: slab0 subtiles 2-3 + bias.
            wt_t = cp.tile([128, KC * NCOL], MDT, name="wt_t")
            nc.sync.dma_start(wt_t[:], wt[:])
            wt_v = wt_t[:].rearrange("p (c n) -> p c n", c=KC)

            def wslice(c):
                return wt_v[:, c, :]

            s0a = x0p.tile([128, HX], MDT, name="s0a")
            nc.sync.dma_start(s0a[:], xt[0:128, 0:HX])
            s0b = x0p.tile([128, HX], MDT, name="s0b")
            nc.scalar.dma_start(s0b[:], xt[0:128, HX:2 * HX])
            bias_sm = cp.tile([128, NCOL], MDT, name="bias_sm")
            nc.scalar.dma_start(bias_sm[:], biasb[:])
            s0v = [s0a[:].rearrange("p (j c m) -> p j c m", j=2, c=KC),
                   s0b[:].rearrange("p (j c m) -> p j c m", j=2, c=KC)]

            xts = {}
            for k in range(1, min(1 + PREFETCH, NSLAB)):
                xt_t = xp.tile([128, NSUB * KC * 128], MDT, tag="xt",
                               name=f"xt_{k}")
                nc.sync.dma_start(xt_t[:], xt[k * 128:(k + 1) * 128, :])
                xts[k] = xt_t

            # Broadcast bias to all 4 subtile rows (fp16 -> fp32) on the
            # (idle at head) Scalar engine.
            biasb_t = cp.tile([128, NSUB, NCOL], F32, name="biasb_t")
            for j in range(NSUB):
                nc.scalar.copy(biasb_t[:, j, :], bias_sm[:])

            # Warm up the PE's HAM clock gate while the weights and slab 0
            # stream in: matmuls on a memset tile, no DMA deps.
            wu_in = cp.tile([128, NCOL], BF16, name="wu_in")
            nc.gpsimd.memset(wu_in[:], 0.125)
            wu_ps = ps_pool.tile([128, NSUB * 512], F32, tag="ps", name="wu_ps")
            for _ in range(N_WARMUP):
                nc.tensor.matmul(
                    wu_ps[:, 0:NCOL], wu_in[:, 0:128], wu_in[:],
                    start=True, stop=True, skip_group_check=True,
                )

            ob = None
            ndb = None
            for k in range(NSLAB):
                kin = k % GROUP
                g = k // GROUP
                if kin == 0:
                    ob = ob_pool.tile([128, GROUP * NSUB * O], F32, tag="ob",
                                      name=f"ob_{g}")
                    # h-major: ndb[:, 0, :] = num plane, ndb[:, 1, :] = den.
                    ndb = nd_pool.tile([128, 2, GROUP * NSUB, O], F32,
                                       tag="ndb", name=f"ndb_{g}")
                kp = k + 1 + PREFETCH
                if kp < NSLAB:
                    xt_t = xp.tile([128, NSUB * KC * 128], MDT, tag="xt",
                                   name=f"xt_{kp}")
                    nc.sync.dma_start(xt_t[:], xt[kp * 128:(kp + 1) * 128, :])
                    xts[kp] = xt_t

                def stat(j, c):
                    if k == 0:
                        return s0v[j // 2][:, j % 2, c, :]
                    return xts[k][:].rearrange(
                        "p (j c m) -> p j c m", j=NSUB, c=KC)[:, j, c, :]

                # h-major planes: [p, h, s, 150]; expert h=0, gate h=1.
                mx = mx_pool.tile([128, 2, NSUB, EO], MDT, tag="mx",
                                  name=f"mx_{k}")
                pe = pe_pool.tile([128, 2, NSUB, EO], MDT, tag="pe",
                                  name=f"pe_{k}")
                if k not in FINE:
                    psum = ps_pool.tile([128, NSUB * 512], F32, tag="ps",
                                        name=f"ps_{k}")
                    for j in range(NSUB):
                        for c in range(KC):
                            nc.tensor.matmul(
                                psum[:, j * 512:j * 512 + NCOL],
                                stat(j, c), wslice(c),
                                start=(c == 0), stop=(c == KC - 1),
                            )
                    # Sole PSUM reader: evict + bias add + narrow to fp16.
                    # (iteration orders match: psum cols are h-major n, the
                    # mx view iterates (s, h, n); only free sizes must agree)
                    nc.vector.tensor_add(
                        mx[:].rearrange("p h s n -> p s h n"),
                        psum[:].rearrange("p (s b) -> p s b", s=NSUB)
                        [:, :, 0:NCOL],
                        biasb_t[:],
                    )
                    nc.scalar.activation(
                        pe[:, 1, :, :], mx[:, 1, :, :],
                        mybir.ActivationFunctionType.Exp,
                    )
                    nc.vector.tensor_mul(
                        pe[:, 0, :, :], mx[:, 0, :, :], pe[:, 1, :, :])
                    # Segmented sum over experts (e contiguous, n = o*E+e):
                    nc.vector.reduce_sum(
                        ndb[:, :, kin * NSUB:(kin + 1) * NSUB, :],
                        pe[:].rearrange("p h s (o e) -> p (h s) o e", o=O),
                        axis=mybir.AxisListType.X,
                    )
                else:
                    # Tail slabs: split psum tiles with per-unit post chains
                    # so the serial chain after the final matmul is one
                    # subtile long (the last slab tapers half+quarter+quarter
                    # and runs its group finals early for subtiles 0..14).
                    units = ([(0, 2), (2, 1), (3, 1)] if k == NSLAB - 1
                             else [(0, 2), (2, 2)])
                    for (j0, nj) in units:
                        psj = ps_pool.tile([128, nj * 512], F32, tag="ps",
                                           name=f"ps_{k}_{j0}",
                                           padded_shape=[128, NSUB * 512])
                        for jj in range(nj):
                            for c in range(KC):
                                nc.tensor.matmul(
                                    psj[:, jj * 512:jj * 512 + NCOL],
                                    stat(j0 + jj, c), wslice(c),
                                    start=(c == 0), stop=(c == KC - 1),
                                )
                        ss = slice(j0, j0 + nj)
                        nc.vector.tensor_add(
                            mx[:, :, ss, :].rearrange("p h s n -> p s h n"),
                            psj[:].rearrange("p (s b) -> p s b", s=nj)
                            [:, :, 0:NCOL],
                            biasb_t[:, ss, :],
                        )
                        nc.scalar.activation(
                            pe[:, 1, ss, :], mx[:, 1, ss, :],
                            mybir.ActivationFunctionType.Exp,
                        )
                        nc.vector.tensor_mul(
                            pe[:, 0, ss, :], mx[:, 0, ss, :], pe[:, 1, ss, :])
                        # (s o) merges (s stride 150 = 15 x o stride 10)
                        nc.vector.reduce_sum(
                            ndb[:, :, kin * NSUB + j0:kin * NSUB + j0 + nj, :],
                            pe[:, :, ss, :]
                            .rearrange("p h s (o e) -> p h (s o) e", o=O),
                            axis=mybir.AxisListType.X,
                        )
                        if k == NSLAB - 1 and (j0, nj) == (2, 1):
                            # Finals for subtiles 0..14 of the group, while
                            # the last subtile's matmuls stream.
                            rden = nd_pool.tile([128, GROUP * NSUB * O], F32,
                                                tag="rden", name=f"rden_{g}")
                            cut = (GROUP * NSUB - 1) * O
                            nc.vector.reciprocal_approx_fast(
                                rden[:, 0:cut],
                                ndb[:, 1, 0:GROUP * NSUB - 1, :]
                                .rearrange("p s o -> p (s o)"))
                            nc.gpsimd.tensor_mul(
                                ob[:, 0:cut],
                                ndb[:, 0, 0:GROUP * NSUB - 1, :]
                                .rearrange("p s o -> p (s o)"),
                                rden[:, 0:cut])
                if kin == GROUP - 1:
                    if k == NSLAB - 1:
                        # Only the final subtile's finals remain here.
                        cut = (GROUP * NSUB - 1) * O
                        nc.vector.reciprocal_approx_fast(
                            rden[:, cut:],
                            ndb[:, 1, GROUP * NSUB - 1:, :]
                            .rearrange("p s o -> p (s o)"))
                        nc.vector.tensor_mul(
                            ob[:, cut:],
                            ndb[:, 0, GROUP * NSUB - 1:, :]
                            .rearrange("p s o -> p (s o)"),
                            rden[:, cut:])
                    else:
                        # Per-group finals; num*rden on the idle GpSimd.
                        rden = nd_pool.tile([128, GROUP * NSUB * O], F32,
                                            tag="rden", name=f"rden_{g}")
                        nc.vector.reciprocal_approx_fast(
                            rden[:],
                            ndb[:, 1, :, :].rearrange("p s o -> p (s o)"))
                        nc.gpsimd.tensor_mul(
                            ob[:],
                            ndb[:, 0, :, :].rearrange("p s o -> p (s o)"),
                            rden[:])
                    g0 = g * GROUP * SLAB
                    # rows r = g0 + p*16 + s (host permutes x to match)
                    nc.scalar.dma_start(
                        out[g0:g0 + GROUP * SLAB, :]
                        .rearrange("(p s) o -> p (s o)", p=128),
                        ob[:],
                    )
    nc.compile()
    return nc


_NC = None


def _get_nc():
    global _NC
    if _NC is None:
        _NC = _build()
    return _NC


def _prep_inputs(x, expert_w, expert_b, gate_w, gate_b):
    # o-major expert columns (n = o*E + e) so the on-chip segmented reduce
    # over experts reads contiguous runs.
    w = np.concatenate([
        np.asarray(expert_w, np.float32).reshape(E, O, K)
        .transpose(1, 0, 2).reshape(EO, K),
        np.asarray(gate_w, np.float32).reshape(E, O, K)
        .transpose(1, 0, 2).reshape(EO, K),
    ], axis=0)                                   # [300, K], col n = o*E + e
    b = np.concatenate([
        np.asarray(expert_b, np.float32).reshape(E, O).T.reshape(EO),
        np.asarray(gate_b, np.float32).reshape(E, O).T.reshape(EO),
    ]).reshape(1, NCOL)
    # wt[q, (c, n)] = w[n, c*128+q]
    wt = np.ascontiguousarray(
        w.reshape(NCOL, KC, 128).transpose(2, 1, 0).astype(NPDT)
        .reshape(128, KC * NCOL))
    biasb = np.ascontiguousarray(
        np.broadcast_to(b, (128, NCOL)).astype(NPDT))
    # Row permutation: within each 2048-row group g of a core, partition p
    # owns rows g*2048 + p*16 + kin*4 + j (slab k = g*4+kin, subtile j).
    # Moving-row index m = p; block layout per slab-row q is (j, c, m).
    x16 = np.asarray(x).astype(NPDT)
    arr = x16.reshape(NCORES, NGRP, 128, GROUP, NSUB, KC, 128)
    #                 core    g     p    kin    j    c   q
    xt = np.ascontiguousarray(arr.transpose(0, 1, 3, 6, 4, 5, 2)) \
        .reshape(NCORES, NSLAB * 128, NSUB * KC * 128)
    in_maps = [{"xt": xt[i], "wt": wt, "biasb": biasb}
               for i in range(NCORES)]
    return in_maps


def _run(in_maps, **kw):
    res = run_bass_kernel_spmd(
        _get_nc(), in_maps, core_ids=list(range(NCORES)), **kw)
    out = np.concatenate([r["out"] for r in res.results], axis=0)
    return out, res


def kernel(x, expert_w, expert_b, gate_w, gate_b):
    in_maps = _prep_inputs(x, expert_w, expert_b, gate_w, gate_b)
    out, _ = _run(in_maps)
    return out


def kernel_traced(x, expert_w, expert_b, gate_w, gate_b, **kw):
    """Like kernel() but returns (out, BassKernelResults) with an NTFF trace."""
    in_maps = _prep_inputs(x, expert_w, expert_b, gate_w, gate_b)
    return _run(in_maps, trace=True, **kw)


# revision 35
# speedup vs baseline: 1.0160x; 1.0160x over previous
"""Trainium2 Bass kernel for nn_MixtureOfExpertsModel (moe_routing).

Computes, for x [65536, 1024] and 10 experts with 15 outputs each:
    miu = x @ expert_w^T + expert_b      (per expert)
    xi  = x @ gate_w^T + gate_b          (per expert)
    out = sum_e softmax_e(xi) * miu      -> [65536, 15]

Strategy: pure data parallel over 8 NeuronCores (8192 rows each); at fp16
the kernel is PE-bound (2.52 GMAC/core -> 153.6k PE cycles = 64 us), so the
design keeps the PE streaming 300-column fp16 matmuls back-to-back at the
~127 ns issue floor and pushes everything else off the critical path:

 * x is repacked on the host into per-slab blocks (512 rows; one contiguous
   8KB run per partition) so every HWDGE load is 128 large descriptors --
   the SDMA per-descriptor fixed cost (~130ns/engine) makes smaller
   descriptors drain far below line rate.
 * head: the first real matmul needs the weights and slab 0, and the Sync
   HWDGE ring starts draining ~2us before the Scalar ring.  Weights and
   slab0's first half ride Sync, slab0's second half plus the (tiny, fp16,
   broadcast on-chip) bias ride Scalar, so group 0 starts ~14us; PE warmup
   matmuls hold the HAM clock gate open until then (a >3.4us idle gap
   would drop the PE back to 1.2GHz for ~3.4us of real matmuls).
 * no bias matmul: the psum->SBUF eviction is a single Vector tensor_add
   per slab that adds the bias and narrows to fp16 (sole PSUM reader).
 * post-processing is batched per 4-subtile slab with h-major planes
   (mx/pe = [p, h, s, 150]) so exp (Scalar) and the product (Vector fp16
   2x) run on contiguous [p, 600] planes; segmented reduce over experts
   (Vector; tensor_reduce has no DVE 2x mode); reciprocal per output GROUP
   (Vector) and the final num*rden on the otherwise idle GpSimd (except
   the last group: Vector, for a shorter tail chain).
 * the last TWO slabs run as two half-slab psum tiles each with per-half
   post chains, halving the serial chain after the final matmul; rows are
   permuted (within each 2048-row group, partition p owns rows
   p*16..p*16+15) so each output store is 128 x 960B descriptors instead
   of 2048 x 60B, keeping the store off the critical tail.
"""

import sys

if "/opt/trn_rl_repo" not in sys.path:
    sys.path.insert(0, "/opt/trn_rl_repo")

import numpy as np

import concourse.bass as bass
import concourse.bacc as bacc
import concourse.tile as tile
import concourse.mybir as mybir
from concourse.bass_utils import run_bass_kernel_spmd

F32 = mybir.dt.float32
FP16 = mybir.dt.float16
BF16 = mybir.dt.bfloat16

MDT = FP16
NPDT = np.float16

BS = 65536
K = 1024
E = 10
O = 15
EO = E * O                # 150
NCOL = 2 * EO             # 300: cols 0..149 = expert (n=o*E+e), 150..299 = gate
NCORES = 8
RPC = BS // NCORES        # rows per core: 8192
KC = K // 128             # 8 contraction chunks
SLAB = 512                # rows per slab = 4 matmul subtiles
NSUB = SLAB // 128        # 4 subtiles per slab
NSLAB = RPC // SLAB       # 16 slabs per core
GROUP = 4                 # slabs per output group (2048 rows per out DMA)
NGRP = NSLAB // GROUP
PREFETCH = 5              # x slabs in flight ahead of compute
N_WARMUP = 32             # PE warmup matmuls (HAM clock-gate release)
FINE = (NSLAB - 2, NSLAB - 1)   # slabs split for a short tail


def _build():
    nc = bacc.Bacc("TRN2", target_bir_lowering=False, debug=False,
                   num_devices=NCORES)
    # xt row k*128+q holds slab k's contiguous (j, c, m) block: j = subtile,
    # c = k-chunk, m = moving-row index p; q = k-chunk partition.
    xt = nc.dram_tensor("xt", [NSLAB * 128, NSUB * KC * 128], MDT,
                        kind="ExternalInput").ap()
    wt = nc.dram_tensor("wt", [128, KC * NCOL], MDT, kind="ExternalInput").ap()
    biasb = nc.dram_tensor("biasb", [128, NCOL], MDT,
                           kind="ExternalInput").ap()
    out = nc.dram_tensor("out", [RPC, O], F32, kind="ExternalOutput").ap()

    HKC = KC // 2
    HX = 2 * KC * 128     # half-slab elements per partition

    with tile.TileContext(nc) as tc:
        with (
            tc.tile_pool(name="const", bufs=1) as cp,
            tc.tile_pool(name="x0", bufs=1) as x0p,
            tc.tile_pool(name="x", bufs=PREFETCH + 2) as xp,
            tc.tile_pool(name="ps", bufs=2, space="PSUM") as ps_pool,
            tc.tile_pool(name="mx", bufs=2) as mx_pool,
            tc.tile_pool(name="pe", bufs=2) as pe_pool,
            tc.tile_pool(name="nd", bufs=2) as nd_pool,
            tc.tile_pool(name="ob", bufs=2) as ob_pool,
        ):
            # Sync ring (drains earliest): weights, slab0 subtiles 0-1, then
            # the slab stream.  Scalar ring: slab0 subtiles 2-3 + bias.
            s0a = x0p.tile([128, HX], MDT, name="s0a")
            nc.sync.dma_start(s0a[:], xt[0:128, 0:HX])
            s0b = x0p.tile([128, HX], MDT, name="s0b")
            nc.sync.dma_start(s0b[:], xt[0:128, HX:2 * HX])
            wt_t = cp.tile([128, KC * NCOL], MDT, name="wt_t")
            nc.scalar.dma_start(wt_t[:], wt[:])
            wt_v = wt_t[:].rearrange("p (c n) -> p c n", c=KC)

            def wslice(c):
                return wt_v[:, c, :]

            bias_sm = cp.tile([128, NCOL], MDT, name="bias_sm")
            nc.scalar.dma_start(bias_sm[:], biasb[:])
            s0v = [s0a[:].rearrange("p (j c m) -> p j c m", j=2, c=KC),
                   s0b[:].rearrange("p (j c m) -> p j c m", j=2, c=KC)]

            xts = {}
            for k in range(1, min(1 + PREFETCH, NSLAB)):
                xt_t = xp.tile([128, NSUB * KC * 128], MDT, tag="xt",
                               name=f"xt_{k}")
                nc.sync.dma_start(xt_t[:], xt[k * 128:(k + 1) * 128, :])
                xts[k] = xt_t

            # Broadcast bias to all 4 subtile rows (fp16 -> fp32) on the
            # (idle at head) Scalar engine.
            biasb_t = cp.tile([128, NSUB, NCOL], F32, name="biasb_t")
            for j in range(NSUB):
                nc.scalar.copy(biasb_t[:, j, :], bias_sm[:])

            # Warm up the PE's HAM clock gate while the weights and slab 0
            # stream in: matmuls on a memset tile, no DMA deps.
            wu_in = cp.tile([128, NCOL], BF16, name="wu_in")
            nc.gpsimd.memset(wu_in[:], 0.125)
            wu_ps = ps_pool.tile([128, NSUB * 512], F32, tag="ps", name="wu_ps")
            for _ in range(N_WARMUP):
                nc.tensor.matmul(
                    wu_ps[:, 0:NCOL], wu_in[:, 0:128], wu_in[:],
                    start=True, stop=True, skip_group_check=True,
                )

            ob = None
            ndb = None
            for k in range(NSLAB):
                kin = k % GROUP
                g = k // GROUP
                if kin == 0:
                    ob = ob_pool.tile([128, GROUP * NSUB * O], F32, tag="ob",
                                      name=f"ob_{g}")
                    # h-major: ndb[:, 0, :] = num plane, ndb[:, 1, :] = den.
                    ndb = nd_pool.tile([128, 2, GROUP * NSUB, O], F32,
                                       tag="ndb", name=f"ndb_{g}")
                kp = k + 1 + PREFETCH
                if kp < NSLAB:
                    xt_t = xp.tile([128, NSUB * KC * 128], MDT, tag="xt",
                                   name=f"xt_{kp}")
                    nc.sync.dma_start(xt_t[:], xt[kp * 128:(kp + 1) * 128, :])
                    xts[kp] = xt_t

                def stat(j, c):
                    if k == 0:
                        return s0v[j // 2][:, j % 2, c, :]
                    return xts[k][:].rearrange(
                        "p (j c m) -> p j c m", j=NSUB, c=KC)[:, j, c, :]

                # h-major planes: [p, h, s, 150]; expert h=0, gate h=1.
                mx = mx_pool.tile([128, 2, NSUB, EO], MDT, tag="mx",
                                  name=f"mx_{k}")
                pe = pe_pool.tile([128, 2, NSUB, EO], MDT, tag="pe",
                                  name=f"pe_{k}")
                if k not in FINE:
                    psum = ps_pool.tile([128, NSUB * 512], F32, tag="ps",
                                        name=f"ps_{k}")
                    for j in range(NSUB):
                        for c in range(KC):
                            nc.tensor.matmul(
                                psum[:, j * 512:j * 512 + NCOL],
                                stat(j, c), wslice(c),
                                start=(c == 0), stop=(c == KC - 1),
                            )
                    # Sole PSUM reader: evict + bias add + narrow to fp16.
                    # (iteration orders match: psum cols are h-major n, the
                    # mx view iterates (s, h, n); only free sizes must agree)
                    nc.vector.tensor_add(
                        mx[:].rearrange("p h s n -> p s h n"),
                        psum[:].rearrange("p (s b) -> p s b", s=NSUB)
                        [:, :, 0:NCOL],
                        biasb_t[:],
                    )
                    nc.scalar.activation(
                        pe[:, 1, :, :], mx[:, 1, :, :],
                        mybir.ActivationFunctionType.Exp,
                    )
                    nc.vector.tensor_mul(
                        pe[:, 0, :, :], mx[:, 0, :, :], pe[:, 1, :, :])
                    # Segmented sum over experts (e contiguous, n = o*E+e):
                    nc.vector.reduce_sum(
                        ndb[:, :, kin * NSUB:(kin + 1) * NSUB, :],
                        pe[:].rearrange("p h s (o e) -> p (h s) o e", o=O),
                        axis=mybir.AxisListType.X,
                    )
                else:
                    # Tail slabs: split psum tiles with per-unit post chains
                    # so the serial chain after the final matmul is one
                    # subtile long (the last slab tapers half+quarter+quarter
                    # and runs its group finals early for subtiles 0..14).
                    units = ([(0, 2), (2, 1), (3, 1)] if k == NSLAB - 1
                             else [(0, 2), (2, 2)])
                    for (j0, nj) in units:
                        psj = ps_pool.tile([128, nj * 512], F32, tag="ps",
                                           name=f"ps_{k}_{j0}",
                                           padded_shape=[128, NSUB * 512])
                        for jj in range(nj):
                            for c in range(KC):
                                nc.tensor.matmul(
                                    psj[:, jj * 512:jj * 512 + NCOL],
                                    stat(j0 + jj, c), wslice(c),
                                    start=(c == 0), stop=(c == KC - 1),
                                )
                        ss = slice(j0, j0 + nj)
                        nc.vector.tensor_add(
                            mx[:, :, ss, :].rearrange("p h s n -> p s h n"),
                            psj[:].rearrange("p (s b) -> p s b", s=nj)
                            [:, :, 0:NCOL],
                            biasb_t[:, ss, :],
                        )
                        nc.scalar.activation(
                            pe[:, 1, ss, :], mx[:, 1, ss, :],
                            mybir.ActivationFunctionType.Exp,
                        )
                        nc.vector.tensor_mul(
                            pe[:, 0, ss, :], mx[:, 0, ss, :], pe[:, 1, ss, :])
                        # (s o) merges (s stride 150 = 15 x o stride 10)
                        nc.vector.reduce_sum(
                            ndb[:, :, kin * NSUB + j0:kin * NSUB + j0 + nj, :],
                            pe[:, :, ss, :]
                            .rearrange("p h s (o e) -> p h (s o) e", o=O),
                            axis=mybir.AxisListType.X,
                        )
                        if k == NSLAB - 1 and (j0, nj) == (2, 1):
                            # Finals for subtiles 0..14 of the group, while
                            # the last subtile's matmuls stream.
                            rden = nd_pool.tile([128, GROUP * NSUB * O], F32,
                                                tag="rden", name=f"rden_{g}")
                            cut = (GROUP * NSUB - 1) * O
                            nc.vector.reciprocal_approx_fast(
                                rden[:, 0:cut],
                                ndb[:, 1, 0:GROUP * NSUB - 1, :]
                                .rearrange("p s o -> p (s o)"))
                            nc.gpsimd.tensor_mul(
                                ob[:, 0:cut],
                                ndb[:, 0, 0:GROUP * NSUB - 1, :]
                                .rearrange("p s o -> p (s o)"),
                                rden[:, 0:cut])
                if kin == GROUP - 1:
                    if k == NSLAB - 1:
                        # Only the final subtile's finals remain here.
                        cut = (GROUP * NSUB - 1) * O
                        nc.vector.reciprocal_approx_fast(
                            rden[:, cut:],
                            ndb[:, 1, GROUP * NSUB - 1:, :]
                            .rearrange("p s o -> p (s o)"))
                        nc.vector.tensor_mul(
                            ob[:, cut:],
                            ndb[:, 0, GROUP * NSUB - 1:, :]
                            .rearrange("p s o -> p (s o)"),
                            rden[:, cut:])
                    else:
                        # Per-group finals; num*rden on the idle GpSimd.
                        rden = nd_pool.tile([128, GROUP * NSUB * O], F32,
                                            tag="rden", name=f"rden_{g}")
                        nc.vector.reciprocal_approx_fast(
                            rden[:],
                            ndb[:, 1, :, :].rearrange("p s o -> p (s o)"))
                        nc.gpsimd.tensor_mul(
                            ob[:],
                            ndb[:, 0, :, :].rearrange("p s o -> p (s o)"),
                            rden[:])
                    g0 = g * GROUP * SLAB
                    # rows r = g0 + p*16 + s (host permutes x to match)
                    nc.scalar.dma_start(
                        out[g0:g0 + GROUP * SLAB, :]
                        .rearrange("(p s) o -> p (s o)", p=128),
                        ob[:],
                    )
    nc.compile()
    return nc


_NC = None


def _get_nc():
    global _NC
    if _NC is None:
        _NC = _build()
    return _NC


def _prep_inputs(x, expert_w, expert_b, gate_w, gate_b):
    # o-major expert columns (n = o*E + e) so the on-chip segmented reduce
    # over experts reads contiguous runs.
    w = np.concatenate([
        np.asarray(expert_w, np.float32).reshape(E, O, K)
        .transpose(1, 0, 2).reshape(EO, K),
        np.asarray(gate_w, np.float32).reshape(E, O, K)
        .transpose(1, 0, 2).reshape(EO, K),
    ], axis=0)                                   # [300, K], col n = o*E + e
    b = np.concatenate([
        np.asarray(expert_b, np.float32).reshape(E, O).T.reshape(EO),
        np.asarray(gate_b, np.float32).reshape(E, O).T.reshape(EO),
    ]).reshape(1, NCOL)
    # wt[q, (c, n)] = w[n, c*128+q]
    wt = np.ascontiguousarray(
        w.reshape(NCOL, KC, 128).transpose(2, 1, 0).astype(NPDT)
        .reshape(128, KC * NCOL))
    biasb = np.ascontiguousarray(
        np.broadcast_to(b, (128, NCOL)).astype(NPDT))
    # Row permutation: within each 2048-row group g of a core, partition p
    # owns rows g*2048 + p*16 + kin*4 + j (slab k = g*4+kin, subtile j).
    # Moving-row index m = p; block layout per slab-row q is (j, c, m).
    x16 = np.asarray(x).astype(NPDT)
    arr = x16.reshape(NCORES, NGRP, 128, GROUP, NSUB, KC, 128)
    #                 core    g     p    kin    j    c   q
    xt = np.ascontiguousarray(arr.transpose(0, 1, 3, 6, 4, 5, 2)) \
        .reshape(NCORES, NSLAB * 128, NSUB * KC * 128)
    in_maps = [{"xt": xt[i], "wt": wt, "biasb": biasb}
               for i in range(NCORES)]
    return in_maps


def _run(in_maps, **kw):
    res = run_bass_kernel_spmd(
        _get_nc(), in_maps, core_ids=list(range(NCORES)), **kw)
    out = np.concatenate([r["out"] for r in res.results], axis=0)
    return out, res


def kernel(x, expert_w, expert_b, gate_w, gate_b):
    in_maps = _prep_inputs(x, expert_w, expert_b, gate_w, gate_b)
    out, _ = _run(in_maps)
    return out


def kernel_traced(x, expert_w, expert_b, gate_w, gate_b, **kw):
    """Like kernel() but returns (out, BassKernelResults) with an NTFF trace."""
    in_maps = _prep_inputs(x, expert_w, expert_b, gate_w, gate_b)
    return _run(in_maps, trace=True, **kw)
